# revision 41
# baseline (speedup 1.0000x reference)
"""Macro-F1 kernel for Trainium2, 8 NeuronCores.

Host side: y_pred is converted to fp16 and counting-sorted by true class
into class-pure 128-row chunks.  Every class is padded to u chunks
(u = 8*ceil(max_c chunks_c / 8), 64 for the 1M/128 input) with sentinel
rows [BIG,0,...,0] whose argmax is exactly class 0, and each core gets
u/8 chunks of every class.  The device program therefore depends only on
u (identical across cores and across inputs with the same u).

Per core, tiles of tk*128 rows laid out [128p, tk, 128c]:
  - DVE: rowmax via pairwise-max f16 tree (64->32->16->8 tensor_tensor
    MAX in the 2x_1p perf mode) + one small tensor_reduce -> rmax f32.
  - rmax2[p,k,0:2] = rmax duplicated (one broadcast tensor_copy).  The
    indicator for the DVE chunks is then ONE tensor_tensor is_lt over
    [128, j, 64, 2] whose src1 AP has innermost [stride 1, count 2]
    (the pair) and a stride-0 middle dim -- this keeps the broadcast
    compare in the 2x_1p perf mode (71ns/chunk vs 143ns/chunk for the
    stride-0-innermost broadcast form; measured).
  - ACT: Sign(rmax - x) for the remaining chunks (~300ns/chunk floor).
  - PE : per chunk one matmul, stationary=anti [128,128] f16,
    moving=ones [128,1], ACCUMULATED per class into one PSUM column
    (start on the class's first chunk, stop on its last).  Output is a
    single [128,128] f32 matrix per core: psum[p, t] =
    128*(u/8) - counts[t, p].
Host: cm[t, p] = 128*u - sum_cores psum[p, t], subtract sentinel
contributions from cm[:, 0], fp64 macro-F1 epilogue.
"""

import sys
import time

if "/opt/trn_rl_repo" not in sys.path:
    sys.path.insert(0, "/opt/trn_rl_repo")

import numpy as np

import concourse.bacc as bacc
import concourse.mybir as mybir
import concourse.tile as tile
from concourse import bass_utils

C = 128
NCORES = 8
TK = 64                  # chunks (of 128 rows) per steady-state tile
RAMP = (4, 8, 16, 32)    # small leading tiles so compute starts early
BIG = np.float16(60000)  # sentinel rows: [BIG, 0, ..., 0] -> argmax == 0
EPS = 1e-12
SJ_FRAC = 0.59           # fraction of chunks compared on DVE (rest ACT)

_CACHE = {}


def _tiles(M):
    out = []
    b = 0
    for tk in RAMP:
        if b + tk > M:
            break
        out.append((b, tk))
        b += tk
    while b < M:
        tk = min(TK, M - b)
        if tk < 16 and out:
            # absorb a tiny tail into the previous tile
            pb, ptk = out.pop()
            out.append((pb, ptk + tk))
            b += tk
        elif tk < TK and tk >= 16:
            h = (tk + 1) // 2
            out.append((b, h))
            out.append((b + h, tk - h))
            b += tk
        else:
            out.append((b, tk))
            b += tk
    return out


def _sched(tiles):
    """DVE chunk count j per tile; chunks [0:j] DVE, [j:tk] ACT."""
    n = len(tiles)
    out = []
    for ti, (b, tk) in enumerate(tiles):
        if ti < len(RAMP):
            j = tk          # ramp: all-DVE so ACT's queue only issues DMAs
        elif ti == n - 1:
            j = tk          # drain: ACT idles, DVE finishes alone
        elif ti == n - 2:
            j = min(tk, 40)
        else:
            j = max(1, round(tk * SJ_FRAC))
        out.append(j)
    return out


def _build(u):
    f32 = mybir.dt.float32
    f16 = mybir.dt.float16
    Alu = mybir.AluOpType
    Act = mybir.ActivationFunctionType

    s = u // 8            # chunks per class per core
    M = C * s             # chunks per core
    R = M * 128

    nc = bacc.Bacc("TRN2", target_bir_lowering=False, debug=False,
                   num_devices=NCORES)
    yp = nc.dram_tensor("yp", [R, C], f16, kind="ExternalInput")
    eye = nc.dram_tensor("eye", [C, C], f16, kind="ExternalInput")
    out = nc.dram_tensor("out", [C, 112], f32, kind="ExternalOutput")
    # last 16 classes go out transposed (PE transpose): the final DMA then
    # spans 16 partitions (16 descriptors instead of 128 -> less drain)
    out2 = nc.dram_tensor("out2", [16, C], f16, kind="ExternalOutput")

    tiles = _tiles(M)
    scheds = _sched(tiles)

    with tile.TileContext(nc) as tc:
        with (
            tc.tile_pool(name="const", bufs=1) as cpool,
            tc.tile_pool(name="xin", bufs=4) as xpool,
            tc.tile_pool(name="anti", bufs=3) as apool,
            tc.tile_pool(name="antib", bufs=3) as bpool,
            tc.tile_pool(name="mtree", bufs=2) as mpool,
            tc.tile_pool(name="small", bufs=4) as spool,
            tc.tile_pool(name="outsb", bufs=1) as opool,
            tc.tile_pool(name="psum", bufs=1, space="PSUM") as psum,
        ):
            ones = cpool.tile([128, 1], f16)
            nc.vector.memset(ones[:], 1.0)
            eyesb = cpool.tile([C, C], f16, name="eyesb", tag="eyesb")
            bank = psum.tile([C, C], f32, name="bank", tag="bank")
            # counts fit exactly in f16 (<= 1024), so the transposed sliver
            # can round-trip through f16
            tbank = psum.tile([16, C], f16, name="tbank", tag="tbank")

            n_evicted = [0]

            def evict(lo, w, tag):
                sb = opool.tile([C, 64], f32, name=f"osb{tag}",
                                tag=f"osb{tag}")
                nc.scalar.copy(sb[:, 0:w], bank[:, lo : lo + w])
                nc.sync.dma_start(out.ap()[:, lo : lo + w], sb[:, 0:w])

            def evict_final():
                sbt = opool.tile([C, 16], f16, name="sbt", tag="sbt")
                nc.scalar.copy(sbt[:], bank[:, 112:128])
                nc.tensor.transpose(tbank[:], sbt[:], eyesb[:])
                sb2 = opool.tile([16, C], f16, name="sb2", tag="sb2")
                nc.scalar.copy(sb2[:], tbank[:])
                nc.sync.dma_start(out2.ap(), sb2[:])

            for ti, (b, tk) in enumerate(tiles):
                x = xpool.tile([128, tk, C], f16, tag="x")
                dma_eng = nc.scalar if ti in (1, 3) else nc.sync
                dma_eng.dma_start(
                    x[:],
                    yp.ap()[b * 128 : (b + tk) * 128, :].rearrange(
                        "(p k) c -> p k c", k=tk
                    ),
                )
                if ti == len(RAMP) - 1:
                    # identity for the final PE transpose: 128-descriptor
                    # DMA, issued late on the otherwise-idle scalar queue
                    nc.scalar.dma_start(eyesb[:], eye.ap())
                m1 = mpool.tile([128, tk, 64], f16, tag="m1")
                nc.vector.tensor_tensor(
                    m1[:], x[:, :, 0:64], x[:, :, 64:128], op=Alu.max
                )
                m2 = mpool.tile([128, tk, 32], f16, tag="m2")
                nc.vector.tensor_tensor(
                    m2[:], m1[:, :, 0:32], m1[:, :, 32:64], op=Alu.max
                )
                m3 = mpool.tile([128, tk, 16], f16, tag="m3")
                nc.vector.tensor_tensor(
                    m3[:], m2[:, :, 0:16], m2[:, :, 16:32], op=Alu.max
                )
                m4 = mpool.tile([128, tk, 8], f16, tag="m4")
                nc.vector.tensor_tensor(
                    m4[:], m3[:, :, 0:8], m3[:, :, 8:16], op=Alu.max
                )
                rmaxf = spool.tile([128, tk], f32, tag="rmax")
                nc.vector.tensor_reduce(
                    rmaxf[:], m4[:], axis=mybir.AxisListType.X, op=Alu.max
                )
                rmax2 = spool.tile([128, tk, 2], f16, tag="rmax2")
                nc.vector.tensor_copy(
                    rmax2[:], rmaxf[:, :, None].broadcast_to([128, tk, 2])
                )

                j = scheds[ti]
                na = tk - j
                anti_d = apool.tile([128, j, C], f16, tag="antid")
                nc.vector.tensor_tensor(
                    anti_d[:].rearrange("p k (q b2) -> p k q b2", b2=2),
                    x[:, 0:j, :].rearrange("p k (q b2) -> p k q b2", b2=2),
                    rmax2[:, 0:j, None, :].broadcast_to([128, j, 64, 2]),
                    op=Alu.is_lt,
                )
                if na:
                    anti_a = bpool.tile([128, na, C], f16, tag="antia")
                    for k in range(na):
                        nc.scalar.activation(
                            anti_a[:, k, :], x[:, j + k, :], Act.Sign,
                            bias=rmaxf[:, j + k : j + k + 1],
                            scale=-1.0,
                        )
                for k in range(tk):
                    G = b + k
                    col = G // s
                    src = anti_d[:, k, :] if k < j else anti_a[:, k - j, :]
                    nc.tensor.matmul(
                        bank[:, col : col + 1], src, ones[:],
                        start=(G % s == 0), stop=(G % s == s - 1),
                    )
                if n_evicted[0] == 0 and b + tk >= 64 * s:
                    evict(0, 64, "lo")
                    n_evicted[0] = 1
                elif n_evicted[0] == 1 and b + tk >= 112 * s:
                    evict(64, 48, "mid")
                    n_evicted[0] = 2
            evict_final()

    nc.compile()
    return nc


def _get_nc(u):
    if u not in _CACHE:
        _CACHE[u] = _build(u)
    return _CACHE[u]


def _layout(y_true):
    """Class-padded layout. Returns (src_cores [8, M*128], n_c, u)."""
    yt = np.asarray(y_true).astype(np.int64).ravel()
    n_c = np.bincount(yt, minlength=C).astype(np.int64)
    chunks_c = (n_c + 127) // 128
    u = max(8, 8 * int(-(-int(chunks_c.max()) // 8)))
    order = np.argsort(yt, kind="stable").astype(np.int64)
    cls_start = np.zeros(C + 1, np.int64)
    cls_start[1:] = np.cumsum(n_c)
    within = np.arange(len(yt), dtype=np.int64) - cls_start[yt[order]]
    pad = np.full(C * u * 128, -1, np.int64)
    pad[yt[order] * (u * 128) + within] = order
    # class-major padded space -> per-core chunk-slot space
    src_cores = (
        pad.reshape(C, NCORES, (u // 8) * 128)
        .transpose(1, 0, 2)
        .reshape(NCORES, -1)
    )
    return src_cores, n_c, u


def _shards(y_pred, src_cores, u):
    """Per-core physical shards in the device's [p, k] tile layout (f16)."""
    yp = np.asarray(y_pred).astype(np.float16)
    M = C * (u // 8)
    tiles = _tiles(M)
    shards = []
    for i in range(NCORES):
        sc = src_cores[i]
        phys = np.empty(M * 128, np.int64)
        for (b, tk) in tiles:
            blk = sc[b * 128 : (b + tk) * 128].reshape(tk, 128)
            phys[b * 128 : (b + tk) * 128] = blk.T.ravel()
        mask = phys < 0
        shard = yp[np.where(mask, 0, phys)]
        if mask.any():
            shard[mask] = np.float16(0.0)
            shard[mask, 0] = BIG
        shards.append(np.ascontiguousarray(shard))
    return shards


def _run(y_pred, y_true, trace=False):
    src_cores, n_c, u = _layout(y_true)
    nc = _get_nc(u)
    shards = _shards(y_pred, src_cores, u)
    eye = np.eye(C, dtype=np.float16)
    in_maps = [{"yp": s, "eye": eye} for s in shards]
    res = None
    for attempt in range(3):
        try:
            res = bass_utils.run_bass_kernel_spmd(
                nc, in_maps, core_ids=list(range(NCORES)), trace=trace
            )
            break
        except Exception:
            if attempt == 2:
                raise
            time.sleep(2.0)

    acc = np.zeros((C, C), np.float64)
    for r in res.results:
        acc[:, 0:112] += r["out"].astype(np.float64)    # psum[p, t<112]
        acc[:, 112:] += r["out2"].astype(np.float64).T  # out2[t-112, p]
    cm = 128.0 * u - acc.T                        # cm[t, p]
    cm[:, 0] -= 128.0 * u - n_c                   # sentinel rows -> pred 0
    diag = np.diagonal(cm)
    precision = diag / (cm.sum(axis=1) + EPS)
    recall = diag / (cm.sum(axis=0) + EPS)
    f1 = 2.0 * precision * recall / (precision + recall + EPS)
    return np.float32(f1.mean()), res


def kernel(y_pred, y_true):
    out, _ = _run(y_pred, y_true, trace=False)
    return out
